# revision 4
# baseline (speedup 1.0000x reference)
"""YOLO decode on 8 NeuronCores — v4: quantized argmax, DVE/ACT/PE only.

Per (image, scale), DMA gathers the [255, HW] map into two SBUF tiles with all
240 class channels contiguous from partition 0 (compute-op partition ranges
must start at a 32-multiple; GPSIMD cannot run tensor ops in this lowering):
    A[128, HW] = [80 a0 cls | 48 a1 cls]
    B[127, HW] = [32 a1 cls | 80 a2 cls | 15 box rows (a,f)]
Encode (per group of 8 chunks):  q = rne_i32(4096*l)  on ACT (Copy, i32 out);
y = 128*q + (2^22+80-k):  A-half on DVE stt (int math), B-half on ACT
(Identity with per-partition f32 bias — exact, verified).  PE transposes each
chunk into PSUM [cell, 256]: class cols 0..240 ((a,k) stride 80), box cols
240..255; the per-scale B stationary carries `step` on its dx/dy diagonal
entries so box offsets arrive pre-scaled.  ONE DVE reduce per group gives
Y = 128*q* + 2^22 + rev*; box cols are staged to SBUF (ACT) so PSUM frees
after the reduce.  All box/cls/mask math then runs ONCE PER IMAGE over a
uniform [p, 30 chunks, 3, .] view (per-chunk anchor/grid constants), and one
output DMA per image writes [128, 540].

Quantization at 1/4096 flips argmax only for top-2 logit gaps < ~2.4e-4
(~1e-3 added rel err vs the 2e-2 gate on the reference distribution).
"""

import sys
from contextlib import ExitStack

import numpy as np

if "/opt/trn_rl_repo" not in sys.path:
    sys.path.insert(0, "/opt/trn_rl_repo")

NCORES = 8
B = 32
BLOC = B // NCORES  # images per core
CCOL = 256  # chunk stride in PSUM columns (240 class + 15 box + 1 pad)
PGRP = 8  # chunks per PSUM group tile
QS = 4096.0  # logit quantization scale
OFS = 1 << 22  # positivity offset, multiple of 128

# (name, H, W, HW, step, thresh, nchunks)
SCALES = [
    ("x13", 13, 13, 169, 32.0, 0.5, 2),
    ("x26", 26, 26, 676, 16.0, 0.5, 6),
    ("x52", 52, 52, 2704, 8.0, 0.9, 22),
]
ROWS_PER_B = sum(hw * 3 for _, _, _, hw, _, _, _ in SCALES)  # 10647
NCH = sum(nch for _, _, _, _, _, _, nch in SCALES)  # 30 chunks per image
CHUNK_BASE = [0, 2, 8]
WB = NCH * 18  # 540 out cols per image
OUT_FLAT = BLOC * 128 * WB
SOFF = {name: CHUNK_BASE[i] * 18 for i, (name, *_r) in enumerate(SCALES)}

# const tile (f32) column layout
C_IDA = 0  # [0:128] pure identity (A transposes)
C_IDB = 128  # [128+127*s : ...+127] per-scale B stationary with step diag
C_GXY = C_IDB + 3 * 127  # [509:569] per-chunk grid*step, [p, 30, 2]
C_ANC = C_GXY + NCH * 2  # [569:749] per-chunk anchors, [p, 30, 3, 2]
C_REVB = C_ANC + NCH * 6  # [749:750] B-tile encode bias (f32)
C_TOT = C_REVB + 1

_PROG_CACHE = {}


def _groups(nch):
    out = []
    g0 = 0
    while g0 < nch:
        out.append((g0, min(PGRP, nch - g0)))
        g0 += PGRP
    return out


def _build_program():
    import concourse.bacc as bacc
    import concourse.mybir as mybir
    from concourse.tile import TileContext

    f32 = mybir.dt.float32
    i32 = mybir.dt.int32
    AL = mybir.AluOpType
    AF = mybir.ActivationFunctionType
    AX = mybir.AxisListType

    nc = bacc.Bacc("TRN2", target_bir_lowering=False, debug=False)

    xin = {}
    for name, _, _, hw, _, _, _ in SCALES:
        xin[name] = nc.dram_tensor(
            name, [BLOC, 255, hw], f32, kind="ExternalInput"
        ).ap()
    c_f32 = nc.dram_tensor("c_f32", [128, C_TOT], f32, kind="ExternalInput").ap()
    c_rev = nc.dram_tensor("c_rev", [128, 1], i32, kind="ExternalInput").ap()
    out = nc.dram_tensor("out", [OUT_FLAT], f32, kind="ExternalOutput").ap()

    with TileContext(nc) as tc, ExitStack() as ctx:
        const = ctx.enter_context(tc.tile_pool(name="const", bufs=1))
        cf = const.tile([128, C_TOT], f32)
        nc.sync.dma_start(cf[:], c_f32[:])
        rev_t = const.tile([128, 1], i32)
        nc.sync.dma_start(rev_t[:], c_rev[:])
        identA = cf[:, C_IDA : C_IDA + 128]
        gxyS = cf[:, C_GXY : C_GXY + NCH * 2].rearrange("p (g j) -> p g j", j=2)
        anchg = cf[:, C_ANC : C_ANC + NCH * 6].rearrange(
            "p (g a j) -> p g a j", a=3, j=2
        )
        revB = cf[:, C_REVB : C_REVB + 1]

        in_pool = ctx.enter_context(tc.tile_pool(name="inp", bufs=2))
        q_pool = ctx.enter_context(tc.tile_pool(name="qp", bufs=2))
        ps_pool = ctx.enter_context(tc.tile_pool(name="ps", bufs=2, space="PSUM"))
        wk = ctx.enter_context(tc.tile_pool(name="wk", bufs=2))
        op = ctx.enter_context(tc.tile_pool(name="op", bufs=2))

        U = {}
        IMG = {}

        def emit_loads(b, s):
            name, Hh, Ww, HW, step, thresh, nch = SCALES[s]
            x = xin[name]
            A = in_pool.tile([128, HW], f32, tag=f"A{s}")
            Bt = in_pool.tile([127, HW], f32, tag=f"B{s}")
            nc.sync.dma_start(A[0:80, :], x[b, 5:85, :])
            nc.sync.dma_start(A[80:128, :], x[b, 90:138, :])
            nc.scalar.dma_start(Bt[0:32, :], x[b, 138:170, :])
            nc.scalar.dma_start(Bt[32:112, :], x[b, 175:255, :])
            # box rows: x[b, a*85+0:5, :] -> B[112+5a : 117+5a]
            # (one 2D DMA per anchor: partition-dim rearranges lower to a
            # linearized AP that clobbers neighboring partitions)
            for a in range(3):
                nc.gpsimd.dma_start(
                    Bt[112 + 5 * a : 117 + 5 * a, :],
                    x[b, 85 * a : 85 * a + 5, :],
                )
            U[(b, s)] = {"A": A, "B": Bt}

        def emit_compute(b, s):
            name, Hh, Ww, HW, step, thresh, nch = SCALES[s]
            u = U[(b, s)]
            A, Bt = u["A"], u["B"]
            if s == 0:
                Y_t = wk.tile([128, NCH * 3], f32, tag="Y")
                bc_t = wk.tile([128, NCH * 15], f32, tag="bc")
                IMG[b] = {"Y": Y_t, "bc": bc_t}
            Y_t = IMG[b]["Y"]
            bc_t = IMG[b]["bc"]
            Yv = Y_t[:].rearrange("p (g a) -> p g a", a=3)
            cb = CHUNK_BASE[s]
            qA = q_pool.tile([128, HW], i32, tag=f"qA{s}")
            qB = q_pool.tile([127, HW], i32, tag=f"qB{s}")
            identB = cf[0:127, 0:127]
            gl = _groups(nch)
            # stage-batched: engine sequencers block on sem waits, so keep
            # each engine's stream free of cross-stage interleaving
            for g0, gch in gl:
                gw = min(HW, (g0 + gch) * 128) - g0 * 128
                cs = slice(g0 * 128, g0 * 128 + gw)
                # encode: q = rne_i32(4096*l)
                nc.scalar.activation(qA[:, cs], A[:, cs], AF.Copy, scale=QS)
                nc.scalar.activation(
                    qB[0:112, cs], Bt[0:112, cs], AF.Copy, scale=QS
                )
            for g0, gch in gl:
                gw = min(HW, (g0 + gch) * 128) - g0 * 128
                cs = slice(g0 * 128, g0 * 128 + gw)
                # y = 128*q + rev: A-half int stt on DVE, B-half on ACT
                # (Identity, per-partition f32 bias; exact for |y| < 2^24)
                nc.vector.scalar_tensor_tensor(
                    out=A[:, cs],
                    in0=qA[:, cs],
                    scalar=128,
                    in1=rev_t[:, 0:1].broadcast_to([128, gw]),
                    op0=AL.mult,
                    op1=AL.add,
                )
                nc.scalar.activation(
                    Bt[0:112, cs],
                    qB[0:112, cs],
                    AF.Identity,
                    bias=revB[0:112, :],
                    scale=128.0,
                )
            Ps = []
            for g0, gch in gl:
                P = ps_pool.tile([128, PGRP * CCOL], f32, tag="P")
                Ps.append(P)
                for c in range(gch):
                    gc = g0 + c
                    cells = min(128, HW - gc * 128)
                    col = gc * 128
                    fo = c * CCOL
                    # both transposes use tile_position (0,0): mixing
                    # base-0/base-64 positions on one PSUM bank is fatal
                    nc.tensor.transpose(
                        P[0:cells, fo : fo + 128],
                        A[:, col : col + cells],
                        identA,
                    )
                    nc.tensor.transpose(
                        P[0:cells, fo + 128 : fo + 255],
                        Bt[:, col : col + cells],
                        identB,
                    )
            for (g0, gch), P in zip(gl, Ps):
                Pg = P[:, :].rearrange("p (g f) -> p g f", f=CCOL)[:, 0:gch]
                # classes at cols 0..240: (a,k) stride 80
                P4c = Pg[:, :, 0:240].rearrange("p g (a k) -> p g a k", k=80)
                gs = slice(cb + g0, cb + g0 + gch)
                # ONE reduce: Y = max_k y  (argmax encoded in low bits)
                nc.vector.tensor_reduce(
                    out=Yv[:, gs, :], in_=P4c, axis=AX.X, op=AL.max
                )
                # stage box cols to SBUF: P frees after the reduce; box math
                # runs once per image
                nc.scalar.activation(
                    out=bc_t[
                        :, (cb + g0) * 15 : (cb + g0 + gch) * 15
                    ].rearrange("p (g f) -> p g f", f=15),
                    in_=Pg[:, :, 240:255],
                    func=AF.Copy,
                )

        def emit_tail(b):
            # per-image box/cls/mask math over uniform [p, 30, 3, .] views
            Y_t = IMG[b]["Y"]
            bc_t = IMG.pop(b)["bc"]
            bcv = bc_t[:].rearrange("p (g a f) -> p g a f", a=3, f=5)
            Ob = op.tile([128, WB], f32, tag="Ob")
            O4 = Ob[:].rearrange("p (g a f) -> p g a f", a=3, f=6)
            O3 = Ob[:].rearrange("p (ga f) -> p ga f", f=6)
            M_t = wk.tile([128, NCH * 3], f32, tag="M")
            fi_t = wk.tile([128, NCH * 3], i32, tag="fi")
            ff_t = wk.tile([128, NCH * 3], f32, tag="ff")
            wh_t = wk.tile([128, 2 * NCH * 3], f32, tag="wh")
            whv = wh_t[:].rearrange("p (j g a) -> p j g a", j=2, a=3)
            u_t = wk.tile([128, 2 * NCH * 3], f32, tag="u")
            uv = u_t[:].rearrange("p (j g a) -> p j g a", j=2, a=3)

            Mv = M_t[:].rearrange("p (g a) -> p g a", a=3)
            nc.vector.tensor_scalar(
                out=Mv[:, 0:8, :],
                in0=bcv[:, 0:8, :, 0],
                scalar1=0.5,
                scalar2=None,
                op0=AL.is_gt,
            )
            nc.vector.tensor_scalar(
                out=Mv[:, 8:30, :],
                in0=bcv[:, 8:30, :, 0],
                scalar1=0.9,
                scalar2=None,
                op0=AL.is_gt,
            )
            # conf: plain copy; the final mask-mult zeroes it where needed
            nc.scalar.activation(
                out=O4[:, :, :, 0], in_=bcv[:, :, :, 0], func=AF.Copy
            )
            for j in range(2):  # j=0: x/w, j=1: y/h
                nc.scalar.activation(
                    out=whv[:, j], in_=bcv[:, :, :, 3 + j], func=AF.Exp
                )
                nc.vector.tensor_tensor(
                    out=whv[:, j],
                    in0=whv[:, j],
                    in1=anchg[:, :, :, j],
                    op=AL.mult,
                )
                # u = dx*step + gx*step, per scale (step differs)
                for s2, (_n2, _h2, _w2, _hw2, step2, _t2, _nc2) in enumerate(
                    SCALES
                ):
                    sl = slice(CHUNK_BASE[s2], CHUNK_BASE[s2] + SCALES[s2][6])
                    w2 = sl.stop - sl.start
                    nc.vector.scalar_tensor_tensor(
                        out=uv[:, j, sl],
                        in0=bcv[:, sl, :, 1 + j],
                        scalar=step2,
                        in1=gxyS[:, sl, j]
                        .unsqueeze(2)
                        .broadcast_to([128, w2, 3]),
                        op0=AL.mult,
                        op1=AL.add,
                    )
                nc.vector.scalar_tensor_tensor(
                    out=O4[:, :, :, 1 + j],
                    in0=whv[:, j],
                    scalar=-0.5,
                    in1=uv[:, j],
                    op0=AL.mult,
                    op1=AL.add,
                )
                nc.vector.tensor_tensor(
                    out=O4[:, :, :, 3 + j],
                    in0=O4[:, :, :, 1 + j],
                    in1=whv[:, j],
                    op=AL.add,
                )
            # cls = 80 - (Y mod 128): trunc via i32 cast with -0.499 bias
            nc.vector.tensor_scalar(
                out=fi_t[:],
                in0=Y_t[:],
                scalar1=1.0 / 128.0,
                scalar2=-0.499,
                op0=AL.mult,
                op1=AL.add,
            )
            nc.vector.tensor_scalar(
                out=ff_t[:], in0=fi_t[:], scalar1=0, scalar2=None, op0=AL.add
            )
            nc.vector.scalar_tensor_tensor(
                out=O3[:, :, 5],
                in0=ff_t[:],
                scalar=128.0,
                in1=Y_t[:],
                op0=AL.mult,
                op1=AL.subtract,
            )
            nc.vector.tensor_scalar(
                out=O3[:, :, 5],
                in0=O3[:, :, 5],
                scalar1=80.0,
                scalar2=None,
                op0=AL.add,
            )
            # zero masked cells
            nc.vector.tensor_tensor(
                out=O3,
                in0=O3,
                in1=M_t[:].unsqueeze(2).broadcast_to([128, NCH * 3, 6]),
                op=AL.mult,
            )
            dst = out[b * 128 * WB : (b + 1) * 128 * WB].rearrange(
                "(p w) -> p w", w=WB
            )
            nc.scalar.dma_start(dst, Ob[:, :])

        # software pipeline: loads(i) | compute(i-1) | tail after each image
        units = [(b, s) for b in range(BLOC) for s in range(len(SCALES))]
        n = len(units)
        for i in range(n + 1):
            if i < n:
                emit_loads(*units[i])
            if 1 <= i <= n:
                bb, ss = units[i - 1]
                emit_compute(bb, ss)
                if ss == len(SCALES) - 1:
                    emit_tail(bb)
    nc.compile()
    return nc


def _host_constants(anchors):
    anchors = np.asarray(anchors, dtype=np.float32).reshape(3, 3, 2)
    rev = np.zeros((128, 1), dtype=np.int32)
    for p in range(0, 80):
        rev[p, 0] = OFS + 80 - p  # a0 cls k = p
    for p in range(80, 128):
        rev[p, 0] = OFS + 80 - (p - 80)  # a1 cls k = p-80 (0..48)
    cf = np.zeros((128, C_TOT), dtype=np.float32)
    cf[:, 0:128] = np.eye(128, dtype=np.float32)
    gxy = np.zeros((128, NCH, 2), dtype=np.float32)
    ang = np.zeros((128, NCH, 3, 2), dtype=np.float32)
    for s, (name, Hh, Ww, HW, step, thresh, nch) in enumerate(SCALES):
        for c in range(nch):
            g = CHUNK_BASE[s] + c
            ang[:, g] = anchors[s][None, :, :]
            for p in range(128):
                hw = c * 128 + p
                if hw < HW:
                    gxy[p, g, 0] = (hw % Ww) * step
                    gxy[p, g, 1] = (hw // Ww) * step
    cf[:, C_GXY : C_GXY + NCH * 2] = gxy.reshape(128, NCH * 2)
    cf[:, C_ANC : C_ANC + NCH * 6] = ang.reshape(128, NCH * 6)
    revb = np.zeros((128, 1), dtype=np.float32)
    for p in range(0, 32):
        revb[p, 0] = OFS + 80 - (p + 48)  # a1 cls k = p+48
    for p in range(32, 112):
        revb[p, 0] = OFS + 80 - (p - 32)  # a2 cls k = p-32
    cf[:, C_REVB : C_REVB + 1] = revb
    return {"c_f32": cf, "c_rev": rev}


def kernel(output13, output26, output52, anchors):
    from concourse.bass_utils import run_bass_kernel_spmd

    if "nc" not in _PROG_CACHE:
        _PROG_CACHE["nc"] = _build_program()
    nc = _PROG_CACHE["nc"]

    consts = _host_constants(np.asarray(anchors, dtype=np.float32))
    xs = {
        "x13": np.asarray(output13, dtype=np.float32).reshape(B, 255, 169),
        "x26": np.asarray(output26, dtype=np.float32).reshape(B, 255, 676),
        "x52": np.asarray(output52, dtype=np.float32).reshape(B, 255, 2704),
    }
    in_maps = []
    for i in range(NCORES):
        m = dict(consts)
        for k, v in xs.items():
            m[k] = np.ascontiguousarray(v[i * BLOC : (i + 1) * BLOC])
        in_maps.append(m)

    _PROG_CACHE["in_maps"] = in_maps
    res = run_bass_kernel_spmd(nc, in_maps, core_ids=list(range(NCORES)))

    full = np.zeros((B * ROWS_PER_B, 6), np.float32)
    scale_full_base = [0, B * 169 * 3, B * 169 * 3 + B * 676 * 3]
    for i in range(NCORES):
        o = np.asarray(res.results[i]["out"]).reshape(BLOC, 128, WB)
        for b in range(BLOC):
            for s, (name, Hh, Ww, HW, step, thresh, nch) in enumerate(SCALES):
                so = SOFF[name]
                seg = o[b][:, so : so + nch * 18].reshape(128, nch, 3, 6)
                rows = seg.transpose(1, 0, 2, 3).reshape(nch * 128 * 3, 6)
                gb = scale_full_base[s] + (i * BLOC + b) * HW * 3
                full[gb : gb + HW * 3] = rows[: HW * 3]
    return full


# revision 5
# speedup vs baseline: 1.0140x; 1.0140x over previous
"""YOLO decode on 8 NeuronCores — v4: quantized argmax, DVE/ACT/PE only.

Per (image, scale), DMA gathers the [255, HW] map into two SBUF tiles with all
240 class channels contiguous from partition 0 (compute-op partition ranges
must start at a 32-multiple; GPSIMD cannot run tensor ops in this lowering):
    A[128, HW] = [80 a0 cls | 48 a1 cls]
    B[127, HW] = [32 a1 cls | 80 a2 cls | 15 box rows (a,f)]
Encode (per group of 8 chunks):  q = rne_i32(4096*l)  on ACT (Copy, i32 out);
y = 128*q + (2^22+80-k):  A-half on DVE stt (int math), B-half on ACT
(Identity with per-partition f32 bias — exact, verified).  PE transposes each
chunk into PSUM [cell, 256]: class cols 0..240 ((a,k) stride 80), box cols
240..255; the per-scale B stationary carries `step` on its dx/dy diagonal
entries so box offsets arrive pre-scaled.  ONE DVE reduce per group gives
Y = 128*q* + 2^22 + rev*; box cols are staged to SBUF (ACT) so PSUM frees
after the reduce.  All box/cls/mask math then runs ONCE PER IMAGE over a
uniform [p, 30 chunks, 3, .] view (per-chunk anchor/grid constants), and one
output DMA per image writes [128, 540].

Quantization at 1/4096 flips argmax only for top-2 logit gaps < ~2.4e-4
(~1e-3 added rel err vs the 2e-2 gate on the reference distribution).
"""

import sys
from contextlib import ExitStack

import numpy as np

if "/opt/trn_rl_repo" not in sys.path:
    sys.path.insert(0, "/opt/trn_rl_repo")

NCORES = 8
B = 32
BLOC = B // NCORES  # images per core
CCOL = 256  # chunk stride in PSUM columns (240 class + 15 box + 1 pad)
PGRP = 8  # chunks per PSUM group tile
QS = 4096.0  # logit quantization scale
OFS = 1 << 22  # positivity offset, multiple of 128

# (name, H, W, HW, step, thresh, nchunks)
SCALES = [
    ("x13", 13, 13, 169, 32.0, 0.5, 2),
    ("x26", 26, 26, 676, 16.0, 0.5, 6),
    ("x52", 52, 52, 2704, 8.0, 0.9, 22),
]
ROWS_PER_B = sum(hw * 3 for _, _, _, hw, _, _, _ in SCALES)  # 10647
NCH = sum(nch for _, _, _, _, _, _, nch in SCALES)  # 30 chunks per image
CHUNK_BASE = [0, 2, 8]
WB = NCH * 18  # 540 out cols per image
OUT_FLAT = BLOC * 128 * WB
SOFF = {name: CHUNK_BASE[i] * 18 for i, (name, *_r) in enumerate(SCALES)}

# const tile (f32) column layout
C_IDA = 0  # [0:128] pure identity (A transposes)
C_IDB = 128  # [128+127*s : ...+127] per-scale B stationary with step diag
C_GXY = C_IDB + 3 * 127  # [509:569] per-chunk grid*step, [p, 30, 2]
C_ANC = C_GXY + NCH * 2  # [569:749] per-chunk anchors, [p, 30, 3, 2]
C_REVB = C_ANC + NCH * 6  # [749:750] B-tile encode bias (f32)
C_TOT = C_REVB + 1

_PROG_CACHE = {}


def _groups(nch):
    out = []
    g0 = 0
    while g0 < nch:
        out.append((g0, min(PGRP, nch - g0)))
        g0 += PGRP
    return out


def _build_program():
    import concourse.bacc as bacc
    import concourse.mybir as mybir
    from concourse.tile import TileContext

    f32 = mybir.dt.float32
    i32 = mybir.dt.int32
    AL = mybir.AluOpType
    AF = mybir.ActivationFunctionType
    AX = mybir.AxisListType

    nc = bacc.Bacc("TRN2", target_bir_lowering=False, debug=False)

    xin = {}
    for name, _, _, hw, _, _, _ in SCALES:
        xin[name] = nc.dram_tensor(
            name, [BLOC, 255, hw], f32, kind="ExternalInput"
        ).ap()
    c_f32 = nc.dram_tensor("c_f32", [128, C_TOT], f32, kind="ExternalInput").ap()
    c_rev = nc.dram_tensor("c_rev", [128, 1], i32, kind="ExternalInput").ap()
    out = nc.dram_tensor("out", [OUT_FLAT], f32, kind="ExternalOutput").ap()

    with TileContext(nc) as tc, ExitStack() as ctx:
        const = ctx.enter_context(tc.tile_pool(name="const", bufs=1))
        cf = const.tile([128, C_TOT], f32)
        nc.sync.dma_start(cf[:], c_f32[:])
        rev_t = const.tile([128, 1], i32)
        nc.sync.dma_start(rev_t[:], c_rev[:])
        identA = cf[:, C_IDA : C_IDA + 128]
        gxyS = cf[:, C_GXY : C_GXY + NCH * 2].rearrange("p (g j) -> p g j", j=2)
        anchg = cf[:, C_ANC : C_ANC + NCH * 6].rearrange(
            "p (g a j) -> p g a j", a=3, j=2
        )
        revB = cf[:, C_REVB : C_REVB + 1]

        in_pool = ctx.enter_context(tc.tile_pool(name="inp", bufs=2))
        q_pool = ctx.enter_context(tc.tile_pool(name="qp", bufs=2))
        ps_pool = ctx.enter_context(tc.tile_pool(name="ps", bufs=2, space="PSUM"))
        wk = ctx.enter_context(tc.tile_pool(name="wk", bufs=2))
        op = ctx.enter_context(tc.tile_pool(name="op", bufs=2))

        U = {}
        IMG = {}

        def emit_loads(b, s):
            name, Hh, Ww, HW, step, thresh, nch = SCALES[s]
            x = xin[name]
            A = in_pool.tile([128, HW], f32, tag=f"A{s}")
            Bt = in_pool.tile([127, HW], f32, tag=f"B{s}")
            nc.sync.dma_start(A[0:80, :], x[b, 5:85, :])
            nc.sync.dma_start(A[80:128, :], x[b, 90:138, :])
            nc.scalar.dma_start(Bt[0:32, :], x[b, 138:170, :])
            nc.scalar.dma_start(Bt[32:112, :], x[b, 175:255, :])
            # box rows: x[b, a*85+0:5, :] -> B[112+5a : 117+5a]
            # (one 2D DMA per anchor: partition-dim rearranges lower to a
            # linearized AP that clobbers neighboring partitions)
            for a in range(3):
                nc.gpsimd.dma_start(
                    Bt[112 + 5 * a : 117 + 5 * a, :],
                    x[b, 85 * a : 85 * a + 5, :],
                )
            U[(b, s)] = {"A": A, "B": Bt}

        def emit_compute(b, s):
            name, Hh, Ww, HW, step, thresh, nch = SCALES[s]
            u = U[(b, s)]
            A, Bt = u["A"], u["B"]
            if s == 2:
                Y_t = wk.tile([128, NCH * 3], f32, tag="Y")
                bc_t = wk.tile([128, NCH * 15], f32, tag="bc")
                IMG[b] = {"Y": Y_t, "bc": bc_t}
            Y_t = IMG[b]["Y"]
            bc_t = IMG[b]["bc"]
            Yv = Y_t[:].rearrange("p (g a) -> p g a", a=3)
            cb = CHUNK_BASE[s]
            qA = q_pool.tile([128, HW], i32, tag=f"qA{s}")
            qB = q_pool.tile([127, HW], i32, tag=f"qB{s}")
            identB = cf[0:127, 0:127]
            gl = _groups(nch)
            # stage-batched: engine sequencers block on sem waits, so keep
            # each engine's stream free of cross-stage interleaving
            for g0, gch in gl:
                gw = min(HW, (g0 + gch) * 128) - g0 * 128
                cs = slice(g0 * 128, g0 * 128 + gw)
                # encode: q = rne_i32(4096*l)
                nc.scalar.activation(qA[:, cs], A[:, cs], AF.Copy, scale=QS)
                nc.scalar.activation(
                    qB[0:112, cs], Bt[0:112, cs], AF.Copy, scale=QS
                )
            for g0, gch in gl:
                gw = min(HW, (g0 + gch) * 128) - g0 * 128
                cs = slice(g0 * 128, g0 * 128 + gw)
                # y = 128*q + rev: A-half int stt on DVE, B-half on ACT
                # (Identity, per-partition f32 bias; exact for |y| < 2^24)
                nc.vector.scalar_tensor_tensor(
                    out=A[:, cs],
                    in0=qA[:, cs],
                    scalar=128,
                    in1=rev_t[:, 0:1].broadcast_to([128, gw]),
                    op0=AL.mult,
                    op1=AL.add,
                )
                nc.scalar.activation(
                    Bt[0:112, cs],
                    qB[0:112, cs],
                    AF.Identity,
                    bias=revB[0:112, :],
                    scale=128.0,
                )
            Ps = []
            for g0, gch in gl:
                P = ps_pool.tile([128, PGRP * CCOL], f32, tag="P")
                Ps.append(P)
                for c in range(gch):
                    gc = g0 + c
                    cells = min(128, HW - gc * 128)
                    col = gc * 128
                    fo = c * CCOL
                    # both transposes use tile_position (0,0): mixing
                    # base-0/base-64 positions on one PSUM bank is fatal
                    nc.tensor.transpose(
                        P[0:cells, fo : fo + 128],
                        A[:, col : col + cells],
                        identA,
                    )
                    nc.tensor.transpose(
                        P[0:cells, fo + 128 : fo + 255],
                        Bt[:, col : col + cells],
                        identB,
                    )
            for (g0, gch), P in zip(gl, Ps):
                Pg = P[:, :].rearrange("p (g f) -> p g f", f=CCOL)[:, 0:gch]
                # classes at cols 0..240: (a,k) stride 80
                P4c = Pg[:, :, 0:240].rearrange("p g (a k) -> p g a k", k=80)
                gs = slice(cb + g0, cb + g0 + gch)
                # ONE reduce: Y = max_k y  (argmax encoded in low bits)
                nc.vector.tensor_reduce(
                    out=Yv[:, gs, :], in_=P4c, axis=AX.X, op=AL.max
                )
                # stage box cols to SBUF: P frees after the reduce; box math
                # runs once per image
                nc.scalar.activation(
                    out=bc_t[
                        :, (cb + g0) * 15 : (cb + g0 + gch) * 15
                    ].rearrange("p (g f) -> p g f", f=15),
                    in_=Pg[:, :, 240:255],
                    func=AF.Copy,
                )

        def emit_tail(b):
            # per-image box/cls/mask math over uniform [p, 30, 3, .] views
            Y_t = IMG[b]["Y"]
            bc_t = IMG.pop(b)["bc"]
            bcv = bc_t[:].rearrange("p (g a f) -> p g a f", a=3, f=5)
            Ob = op.tile([128, WB], f32, tag="Ob")
            O4 = Ob[:].rearrange("p (g a f) -> p g a f", a=3, f=6)
            O3 = Ob[:].rearrange("p (ga f) -> p ga f", f=6)
            M_t = wk.tile([128, NCH * 3], f32, tag="M")
            fi_t = wk.tile([128, NCH * 3], i32, tag="fi")
            ff_t = wk.tile([128, NCH * 3], f32, tag="ff")
            wh_t = wk.tile([128, 2 * NCH * 3], f32, tag="wh")
            whv = wh_t[:].rearrange("p (j g a) -> p j g a", j=2, a=3)
            u_t = wk.tile([128, 2 * NCH * 3], f32, tag="u")
            uv = u_t[:].rearrange("p (j g a) -> p j g a", j=2, a=3)

            Mv = M_t[:].rearrange("p (g a) -> p g a", a=3)
            nc.vector.tensor_scalar(
                out=Mv[:, 0:8, :],
                in0=bcv[:, 0:8, :, 0],
                scalar1=0.5,
                scalar2=None,
                op0=AL.is_gt,
            )
            nc.vector.tensor_scalar(
                out=Mv[:, 8:30, :],
                in0=bcv[:, 8:30, :, 0],
                scalar1=0.9,
                scalar2=None,
                op0=AL.is_gt,
            )
            # conf: plain copy; the final mask-mult zeroes it where needed
            nc.scalar.activation(
                out=O4[:, :, :, 0], in_=bcv[:, :, :, 0], func=AF.Copy
            )
            for j in range(2):  # j=0: x/w, j=1: y/h
                nc.scalar.activation(
                    out=whv[:, j], in_=bcv[:, :, :, 3 + j], func=AF.Exp
                )
                nc.vector.tensor_tensor(
                    out=whv[:, j],
                    in0=whv[:, j],
                    in1=anchg[:, :, :, j],
                    op=AL.mult,
                )
                # u = dx*step + gx*step, per scale (step differs)
                for s2, (_n2, _h2, _w2, _hw2, step2, _t2, _nc2) in enumerate(
                    SCALES
                ):
                    sl = slice(CHUNK_BASE[s2], CHUNK_BASE[s2] + SCALES[s2][6])
                    w2 = sl.stop - sl.start
                    nc.vector.scalar_tensor_tensor(
                        out=uv[:, j, sl],
                        in0=bcv[:, sl, :, 1 + j],
                        scalar=step2,
                        in1=gxyS[:, sl, j]
                        .unsqueeze(2)
                        .broadcast_to([128, w2, 3]),
                        op0=AL.mult,
                        op1=AL.add,
                    )
                nc.vector.scalar_tensor_tensor(
                    out=O4[:, :, :, 1 + j],
                    in0=whv[:, j],
                    scalar=-0.5,
                    in1=uv[:, j],
                    op0=AL.mult,
                    op1=AL.add,
                )
                nc.vector.tensor_tensor(
                    out=O4[:, :, :, 3 + j],
                    in0=O4[:, :, :, 1 + j],
                    in1=whv[:, j],
                    op=AL.add,
                )
            # cls = 80 - (Y mod 128): trunc via i32 cast with -0.499 bias
            nc.vector.tensor_scalar(
                out=fi_t[:],
                in0=Y_t[:],
                scalar1=1.0 / 128.0,
                scalar2=-0.499,
                op0=AL.mult,
                op1=AL.add,
            )
            nc.vector.tensor_scalar(
                out=ff_t[:], in0=fi_t[:], scalar1=0, scalar2=None, op0=AL.add
            )
            nc.vector.scalar_tensor_tensor(
                out=O3[:, :, 5],
                in0=ff_t[:],
                scalar=128.0,
                in1=Y_t[:],
                op0=AL.mult,
                op1=AL.subtract,
            )
            nc.vector.tensor_scalar(
                out=O3[:, :, 5],
                in0=O3[:, :, 5],
                scalar1=80.0,
                scalar2=None,
                op0=AL.add,
            )
            # zero masked cells
            nc.vector.tensor_tensor(
                out=O3,
                in0=O3,
                in1=M_t[:].unsqueeze(2).broadcast_to([128, NCH * 3, 6]),
                op=AL.mult,
            )
            dst = out[b * 128 * WB : (b + 1) * 128 * WB].rearrange(
                "(p w) -> p w", w=WB
            )
            nc.scalar.dma_start(dst, Ob[:, :])

        # software pipeline: loads(i) | compute(i-1) | tail after each image
        units = [(b, s) for b in range(BLOC) for s in (2, 1, 0)]
        n = len(units)
        for i in range(n + 1):
            if i < n:
                emit_loads(*units[i])
            if 1 <= i <= n:
                bb, ss = units[i - 1]
                emit_compute(bb, ss)
                if ss == 0:
                    emit_tail(bb)
    nc.compile()
    return nc


def _host_constants(anchors):
    anchors = np.asarray(anchors, dtype=np.float32).reshape(3, 3, 2)
    rev = np.zeros((128, 1), dtype=np.int32)
    for p in range(0, 80):
        rev[p, 0] = OFS + 80 - p  # a0 cls k = p
    for p in range(80, 128):
        rev[p, 0] = OFS + 80 - (p - 80)  # a1 cls k = p-80 (0..48)
    cf = np.zeros((128, C_TOT), dtype=np.float32)
    cf[:, 0:128] = np.eye(128, dtype=np.float32)
    gxy = np.zeros((128, NCH, 2), dtype=np.float32)
    ang = np.zeros((128, NCH, 3, 2), dtype=np.float32)
    for s, (name, Hh, Ww, HW, step, thresh, nch) in enumerate(SCALES):
        for c in range(nch):
            g = CHUNK_BASE[s] + c
            ang[:, g] = anchors[s][None, :, :]
            for p in range(128):
                hw = c * 128 + p
                if hw < HW:
                    gxy[p, g, 0] = (hw % Ww) * step
                    gxy[p, g, 1] = (hw // Ww) * step
    cf[:, C_GXY : C_GXY + NCH * 2] = gxy.reshape(128, NCH * 2)
    cf[:, C_ANC : C_ANC + NCH * 6] = ang.reshape(128, NCH * 6)
    revb = np.zeros((128, 1), dtype=np.float32)
    for p in range(0, 32):
        revb[p, 0] = OFS + 80 - (p + 48)  # a1 cls k = p+48
    for p in range(32, 112):
        revb[p, 0] = OFS + 80 - (p - 32)  # a2 cls k = p-32
    cf[:, C_REVB : C_REVB + 1] = revb
    return {"c_f32": cf, "c_rev": rev}


def kernel(output13, output26, output52, anchors):
    from concourse.bass_utils import run_bass_kernel_spmd

    if "nc" not in _PROG_CACHE:
        _PROG_CACHE["nc"] = _build_program()
    nc = _PROG_CACHE["nc"]

    consts = _host_constants(np.asarray(anchors, dtype=np.float32))
    xs = {
        "x13": np.asarray(output13, dtype=np.float32).reshape(B, 255, 169),
        "x26": np.asarray(output26, dtype=np.float32).reshape(B, 255, 676),
        "x52": np.asarray(output52, dtype=np.float32).reshape(B, 255, 2704),
    }
    in_maps = []
    for i in range(NCORES):
        m = dict(consts)
        for k, v in xs.items():
            m[k] = np.ascontiguousarray(v[i * BLOC : (i + 1) * BLOC])
        in_maps.append(m)

    _PROG_CACHE["in_maps"] = in_maps
    res = run_bass_kernel_spmd(nc, in_maps, core_ids=list(range(NCORES)))

    full = np.zeros((B * ROWS_PER_B, 6), np.float32)
    scale_full_base = [0, B * 169 * 3, B * 169 * 3 + B * 676 * 3]
    for i in range(NCORES):
        o = np.asarray(res.results[i]["out"]).reshape(BLOC, 128, WB)
        for b in range(BLOC):
            for s, (name, Hh, Ww, HW, step, thresh, nch) in enumerate(SCALES):
                so = SOFF[name]
                seg = o[b][:, so : so + nch * 18].reshape(128, nch, 3, 6)
                rows = seg.transpose(1, 0, 2, 3).reshape(nch * 128 * 3, 6)
                gb = scale_full_base[s] + (i * BLOC + b) * HW * 3
                full[gb : gb + HW * 3] = rows[: HW * 3]
    return full


# revision 6
# speedup vs baseline: 1.0524x; 1.0378x over previous
"""YOLO decode on 8 NeuronCores — v4: quantized argmax, DVE/ACT/PE only.

Per (image, scale), DMA gathers the [255, HW] map into two SBUF tiles with all
240 class channels contiguous from partition 0 (compute-op partition ranges
must start at a 32-multiple; GPSIMD cannot run tensor ops in this lowering):
    A[128, HW] = [80 a0 cls | 48 a1 cls]
    B[127, HW] = [32 a1 cls | 80 a2 cls | 15 box rows (a,f)]
Encode (per group of 8 chunks):  q = rne_i32(4096*l)  on ACT (Copy, i32 out);
y = 128*q + (2^22+80-k):  A-half on DVE stt (int math), B-half on ACT
(Identity with per-partition f32 bias — exact, verified).  PE transposes each
chunk into PSUM [cell, 256]: class cols 0..240 ((a,k) stride 80), box cols
240..255; the per-scale B stationary carries `step` on its dx/dy diagonal
entries so box offsets arrive pre-scaled.  ONE DVE reduce per group gives
Y = 128*q* + 2^22 + rev*; box cols are staged to SBUF (ACT) so PSUM frees
after the reduce.  All box/cls/mask math then runs ONCE PER IMAGE over a
uniform [p, 30 chunks, 3, .] view (per-chunk anchor/grid constants), and one
output DMA per image writes [128, 540].

Quantization at 1/4096 flips argmax only for top-2 logit gaps < ~2.4e-4
(~1e-3 added rel err vs the 2e-2 gate on the reference distribution).
"""

import sys
from contextlib import ExitStack

import numpy as np

if "/opt/trn_rl_repo" not in sys.path:
    sys.path.insert(0, "/opt/trn_rl_repo")

NCORES = 8
B = 32
BLOC = B // NCORES  # images per core
CCOL = 256  # chunk stride in PSUM columns (240 class + 15 box + 1 pad)
PGRP = 8  # chunks per PSUM group tile
QS = 4096.0  # logit quantization scale
OFS = 1 << 22  # positivity offset, multiple of 128

# (name, H, W, HW, step, thresh, nchunks)
SCALES = [
    ("x13", 13, 13, 169, 32.0, 0.5, 2),
    ("x26", 26, 26, 676, 16.0, 0.5, 6),
    ("x52", 52, 52, 2704, 8.0, 0.9, 22),
]
ROWS_PER_B = sum(hw * 3 for _, _, _, hw, _, _, _ in SCALES)  # 10647
NCH = sum(nch for _, _, _, _, _, _, nch in SCALES)  # 30 chunks per image
CHUNK_BASE = [0, 2, 8]
WB = NCH * 18  # 540 out cols per image
OUT_FLAT = BLOC * 128 * WB
SOFF = {name: CHUNK_BASE[i] * 18 for i, (name, *_r) in enumerate(SCALES)}

# const tile (f32) column layout
C_IDA = 0  # [0:128] pure identity (A transposes)
C_IDB = 128  # [128+127*s : ...+127] per-scale B stationary with step diag
C_GXY = C_IDB + 3 * 127  # [509:569] per-chunk grid*step, [p, 30, 2]
C_ANC = C_GXY + NCH * 2  # [569:749] per-chunk anchors, [p, 30, 3, 2]
C_REVB = C_ANC + NCH * 6  # [749:750] B-tile encode bias (f32)
C_TOT = C_REVB + 1

_PROG_CACHE = {}


def _groups(nch):
    out = []
    g0 = 0
    while g0 < nch:
        out.append((g0, min(PGRP, nch - g0)))
        g0 += PGRP
    return out


def _build_program():
    import concourse.bacc as bacc
    import concourse.mybir as mybir
    from concourse.tile import TileContext

    f32 = mybir.dt.float32
    i32 = mybir.dt.int32
    AL = mybir.AluOpType
    AF = mybir.ActivationFunctionType
    AX = mybir.AxisListType

    nc = bacc.Bacc("TRN2", target_bir_lowering=False, debug=False)

    xin = {}
    for name, _, _, hw, _, _, _ in SCALES:
        xin[name] = nc.dram_tensor(
            name, [BLOC, 255, hw], f32, kind="ExternalInput"
        ).ap()
    c_f32 = nc.dram_tensor("c_f32", [128, C_TOT], f32, kind="ExternalInput").ap()
    c_rev = nc.dram_tensor("c_rev", [128, 1], i32, kind="ExternalInput").ap()
    out = nc.dram_tensor("out", [OUT_FLAT], f32, kind="ExternalOutput").ap()

    with TileContext(nc) as tc, ExitStack() as ctx:
        const = ctx.enter_context(tc.tile_pool(name="const", bufs=1))
        cf = const.tile([128, C_TOT], f32)
        nc.sync.dma_start(cf[:], c_f32[:])
        rev_t = const.tile([128, 1], i32)
        nc.sync.dma_start(rev_t[:], c_rev[:])
        identA = cf[:, C_IDA : C_IDA + 128]
        gxyS = cf[:, C_GXY : C_GXY + NCH * 2].rearrange("p (g j) -> p g j", j=2)
        anchg = cf[:, C_ANC : C_ANC + NCH * 6].rearrange(
            "p (g a j) -> p g a j", a=3, j=2
        )
        revB = cf[:, C_REVB : C_REVB + 1]

        in_pool = ctx.enter_context(tc.tile_pool(name="inp", bufs=2))
        q_pool = ctx.enter_context(tc.tile_pool(name="qp", bufs=2))
        ps_pool = ctx.enter_context(tc.tile_pool(name="ps", bufs=2, space="PSUM"))
        wk = ctx.enter_context(tc.tile_pool(name="wk", bufs=2))
        op = ctx.enter_context(tc.tile_pool(name="op", bufs=2))

        U = {}
        IMG = {}

        def emit_loads(b, s):
            name, Hh, Ww, HW, step, thresh, nch = SCALES[s]
            x = xin[name]
            A = in_pool.tile([128, HW], f32, tag=f"A{s}")
            Bt = in_pool.tile([127, HW], f32, tag=f"B{s}")
            nc.sync.dma_start(A[0:80, :], x[b, 5:85, :])
            nc.sync.dma_start(A[80:128, :], x[b, 90:138, :])
            nc.scalar.dma_start(Bt[0:32, :], x[b, 138:170, :])
            nc.scalar.dma_start(Bt[32:112, :], x[b, 175:255, :])
            # box rows: x[b, a*85+0:5, :] -> B[112+5a : 117+5a]
            # (one 2D DMA per anchor: partition-dim rearranges lower to a
            # linearized AP that clobbers neighboring partitions)
            for a in range(3):
                nc.gpsimd.dma_start(
                    Bt[112 + 5 * a : 117 + 5 * a, :],
                    x[b, 85 * a : 85 * a + 5, :],
                )
            U[(b, s)] = {"A": A, "B": Bt}

        def emit_compute(b, s):
            name, Hh, Ww, HW, step, thresh, nch = SCALES[s]
            u = U[(b, s)]
            A, Bt = u["A"], u["B"]
            if b not in IMG:
                Y_t = wk.tile([128, NCH * 3], f32, tag="Y")
                bc_t = wk.tile([128, NCH * 15], f32, tag="bc")
                IMG[b] = {"Y": Y_t, "bc": bc_t, "n": 0}
            IMG[b]["n"] += 1
            Y_t = IMG[b]["Y"]
            bc_t = IMG[b]["bc"]
            Yv = Y_t[:].rearrange("p (g a) -> p g a", a=3)
            cb = CHUNK_BASE[s]
            qA = q_pool.tile([128, HW], i32, tag=f"qA{s}")
            qB = q_pool.tile([127, HW], i32, tag=f"qB{s}")
            identB = cf[0:127, 0:127]
            gl = _groups(nch)
            # stage-batched: engine sequencers block on sem waits, so keep
            # each engine's stream free of cross-stage interleaving
            for g0, gch in gl:
                gw = min(HW, (g0 + gch) * 128) - g0 * 128
                cs = slice(g0 * 128, g0 * 128 + gw)
                # encode: q = rne_i32(4096*l)
                nc.scalar.activation(qA[:, cs], A[:, cs], AF.Copy, scale=QS)
                nc.scalar.activation(
                    qB[0:112, cs], Bt[0:112, cs], AF.Copy, scale=QS
                )
            for g0, gch in gl:
                gw = min(HW, (g0 + gch) * 128) - g0 * 128
                cs = slice(g0 * 128, g0 * 128 + gw)
                # y = 128*q + rev: A-half int stt on DVE, B-half on ACT
                # (Identity, per-partition f32 bias; exact for |y| < 2^24)
                nc.vector.scalar_tensor_tensor(
                    out=A[:, cs],
                    in0=qA[:, cs],
                    scalar=128,
                    in1=rev_t[:, 0:1].broadcast_to([128, gw]),
                    op0=AL.mult,
                    op1=AL.add,
                )
                nc.scalar.activation(
                    Bt[0:112, cs],
                    qB[0:112, cs],
                    AF.Identity,
                    bias=revB[0:112, :],
                    scale=128.0,
                )
            Ps = []
            for g0, gch in gl:
                P = ps_pool.tile([128, PGRP * CCOL], f32, tag="P")
                Ps.append(P)
                for c in range(gch):
                    gc = g0 + c
                    cells = min(128, HW - gc * 128)
                    col = gc * 128
                    fo = c * CCOL
                    # both transposes use tile_position (0,0): mixing
                    # base-0/base-64 positions on one PSUM bank is fatal
                    nc.tensor.transpose(
                        P[0:cells, fo : fo + 128],
                        A[:, col : col + cells],
                        identA,
                    )
                    nc.tensor.transpose(
                        P[0:cells, fo + 128 : fo + 255],
                        Bt[:, col : col + cells],
                        identB,
                    )
            for (g0, gch), P in zip(gl, Ps):
                Pg = P[:, :].rearrange("p (g f) -> p g f", f=CCOL)[:, 0:gch]
                # classes at cols 0..240: (a,k) stride 80
                P4c = Pg[:, :, 0:240].rearrange("p g (a k) -> p g a k", k=80)
                gs = slice(cb + g0, cb + g0 + gch)
                # ONE reduce: Y = max_k y  (argmax encoded in low bits)
                nc.vector.tensor_reduce(
                    out=Yv[:, gs, :], in_=P4c, axis=AX.X, op=AL.max
                )
                # stage box cols to SBUF: P frees after the reduce; box math
                # runs once per image
                nc.scalar.activation(
                    out=bc_t[
                        :, (cb + g0) * 15 : (cb + g0 + gch) * 15
                    ].rearrange("p (g f) -> p g f", f=15),
                    in_=Pg[:, :, 240:255],
                    func=AF.Copy,
                )

        def emit_tail(b):
            # per-image box/cls/mask math over uniform [p, 30, 3, .] views
            Y_t = IMG[b]["Y"]
            bc_t = IMG.pop(b)["bc"]
            bcv = bc_t[:].rearrange("p (g a f) -> p g a f", a=3, f=5)
            Ob = op.tile([128, WB], f32, tag="Ob")
            O4 = Ob[:].rearrange("p (g a f) -> p g a f", a=3, f=6)
            O3 = Ob[:].rearrange("p (ga f) -> p ga f", f=6)
            M_t = wk.tile([128, NCH * 3], f32, tag="M")
            fi_t = wk.tile([128, NCH * 3], i32, tag="fi")
            ff_t = wk.tile([128, NCH * 3], f32, tag="ff")
            wh_t = wk.tile([128, 2 * NCH * 3], f32, tag="wh")
            whv = wh_t[:].rearrange("p (j g a) -> p j g a", j=2, a=3)
            u_t = wk.tile([128, 2 * NCH * 3], f32, tag="u")
            uv = u_t[:].rearrange("p (j g a) -> p j g a", j=2, a=3)

            Mv = M_t[:].rearrange("p (g a) -> p g a", a=3)
            nc.vector.tensor_scalar(
                out=Mv[:, 0:8, :],
                in0=bcv[:, 0:8, :, 0],
                scalar1=0.5,
                scalar2=None,
                op0=AL.is_gt,
            )
            nc.vector.tensor_scalar(
                out=Mv[:, 8:30, :],
                in0=bcv[:, 8:30, :, 0],
                scalar1=0.9,
                scalar2=None,
                op0=AL.is_gt,
            )
            # conf: plain copy; the final mask-mult zeroes it where needed
            nc.scalar.activation(
                out=O4[:, :, :, 0], in_=bcv[:, :, :, 0], func=AF.Copy
            )
            for j in range(2):  # j=0: x/w, j=1: y/h
                nc.scalar.activation(
                    out=whv[:, j], in_=bcv[:, :, :, 3 + j], func=AF.Exp
                )
                nc.vector.tensor_tensor(
                    out=whv[:, j],
                    in0=whv[:, j],
                    in1=anchg[:, :, :, j],
                    op=AL.mult,
                )
                # u = dx*step + gx*step, per scale (step differs)
                for s2, (_n2, _h2, _w2, _hw2, step2, _t2, _nc2) in enumerate(
                    SCALES
                ):
                    sl = slice(CHUNK_BASE[s2], CHUNK_BASE[s2] + SCALES[s2][6])
                    w2 = sl.stop - sl.start
                    nc.vector.scalar_tensor_tensor(
                        out=uv[:, j, sl],
                        in0=bcv[:, sl, :, 1 + j],
                        scalar=step2,
                        in1=gxyS[:, sl, j]
                        .unsqueeze(2)
                        .broadcast_to([128, w2, 3]),
                        op0=AL.mult,
                        op1=AL.add,
                    )
                nc.vector.scalar_tensor_tensor(
                    out=O4[:, :, :, 1 + j],
                    in0=whv[:, j],
                    scalar=-0.5,
                    in1=uv[:, j],
                    op0=AL.mult,
                    op1=AL.add,
                )
                nc.vector.tensor_tensor(
                    out=O4[:, :, :, 3 + j],
                    in0=O4[:, :, :, 1 + j],
                    in1=whv[:, j],
                    op=AL.add,
                )
            # cls = 80 - (Y mod 128): trunc via i32 cast with -0.499 bias
            nc.vector.tensor_scalar(
                out=fi_t[:],
                in0=Y_t[:],
                scalar1=1.0 / 128.0,
                scalar2=-0.499,
                op0=AL.mult,
                op1=AL.add,
            )
            nc.vector.tensor_scalar(
                out=ff_t[:], in0=fi_t[:], scalar1=0, scalar2=None, op0=AL.add
            )
            nc.vector.scalar_tensor_tensor(
                out=O3[:, :, 5],
                in0=ff_t[:],
                scalar=128.0,
                in1=Y_t[:],
                op0=AL.mult,
                op1=AL.subtract,
            )
            nc.vector.tensor_scalar(
                out=O3[:, :, 5],
                in0=O3[:, :, 5],
                scalar1=80.0,
                scalar2=None,
                op0=AL.add,
            )
            # zero masked cells
            nc.vector.tensor_tensor(
                out=O3,
                in0=O3,
                in1=M_t[:].unsqueeze(2).broadcast_to([128, NCH * 3, 6]),
                op=AL.mult,
            )
            dst = out[b * 128 * WB : (b + 1) * 128 * WB].rearrange(
                "(p w) -> p w", w=WB
            )
            nc.scalar.dma_start(dst, Ob[:, :])

        # software pipeline: loads(i) | compute(i-1) | tail after each image
        units = [(0, 0), (0, 1), (0, 2)] + [
            (b, s) for b in range(1, BLOC) for s in (2, 0, 1)
        ]
        n = len(units)
        for i in range(n + 1):
            if i < n:
                emit_loads(*units[i])
            if 1 <= i <= n:
                bb, ss = units[i - 1]
                emit_compute(bb, ss)
                if IMG[bb]["n"] == len(SCALES):
                    emit_tail(bb)
    nc.compile()
    return nc


def _host_constants(anchors):
    anchors = np.asarray(anchors, dtype=np.float32).reshape(3, 3, 2)
    rev = np.zeros((128, 1), dtype=np.int32)
    for p in range(0, 80):
        rev[p, 0] = OFS + 80 - p  # a0 cls k = p
    for p in range(80, 128):
        rev[p, 0] = OFS + 80 - (p - 80)  # a1 cls k = p-80 (0..48)
    cf = np.zeros((128, C_TOT), dtype=np.float32)
    cf[:, 0:128] = np.eye(128, dtype=np.float32)
    gxy = np.zeros((128, NCH, 2), dtype=np.float32)
    ang = np.zeros((128, NCH, 3, 2), dtype=np.float32)
    for s, (name, Hh, Ww, HW, step, thresh, nch) in enumerate(SCALES):
        for c in range(nch):
            g = CHUNK_BASE[s] + c
            ang[:, g] = anchors[s][None, :, :]
            for p in range(128):
                hw = c * 128 + p
                if hw < HW:
                    gxy[p, g, 0] = (hw % Ww) * step
                    gxy[p, g, 1] = (hw // Ww) * step
    cf[:, C_GXY : C_GXY + NCH * 2] = gxy.reshape(128, NCH * 2)
    cf[:, C_ANC : C_ANC + NCH * 6] = ang.reshape(128, NCH * 6)
    revb = np.zeros((128, 1), dtype=np.float32)
    for p in range(0, 32):
        revb[p, 0] = OFS + 80 - (p + 48)  # a1 cls k = p+48
    for p in range(32, 112):
        revb[p, 0] = OFS + 80 - (p - 32)  # a2 cls k = p-32
    cf[:, C_REVB : C_REVB + 1] = revb
    return {"c_f32": cf, "c_rev": rev}


def kernel(output13, output26, output52, anchors):
    from concourse.bass_utils import run_bass_kernel_spmd

    if "nc" not in _PROG_CACHE:
        _PROG_CACHE["nc"] = _build_program()
    nc = _PROG_CACHE["nc"]

    consts = _host_constants(np.asarray(anchors, dtype=np.float32))
    xs = {
        "x13": np.asarray(output13, dtype=np.float32).reshape(B, 255, 169),
        "x26": np.asarray(output26, dtype=np.float32).reshape(B, 255, 676),
        "x52": np.asarray(output52, dtype=np.float32).reshape(B, 255, 2704),
    }
    in_maps = []
    for i in range(NCORES):
        m = dict(consts)
        for k, v in xs.items():
            m[k] = np.ascontiguousarray(v[i * BLOC : (i + 1) * BLOC])
        in_maps.append(m)

    _PROG_CACHE["in_maps"] = in_maps
    res = run_bass_kernel_spmd(nc, in_maps, core_ids=list(range(NCORES)))

    full = np.zeros((B * ROWS_PER_B, 6), np.float32)
    scale_full_base = [0, B * 169 * 3, B * 169 * 3 + B * 676 * 3]
    for i in range(NCORES):
        o = np.asarray(res.results[i]["out"]).reshape(BLOC, 128, WB)
        for b in range(BLOC):
            for s, (name, Hh, Ww, HW, step, thresh, nch) in enumerate(SCALES):
                so = SOFF[name]
                seg = o[b][:, so : so + nch * 18].reshape(128, nch, 3, 6)
                rows = seg.transpose(1, 0, 2, 3).reshape(nch * 128 * 3, 6)
                gb = scale_full_base[s] + (i * BLOC + b) * HW * 3
                full[gb : gb + HW * 3] = rows[: HW * 3]
    return full


# revision 7
# speedup vs baseline: 1.0604x; 1.0076x over previous
"""YOLO decode on 8 NeuronCores — v4: quantized argmax, DVE/ACT/PE only.

Per (image, scale), DMA gathers the [255, HW] map into two SBUF tiles with all
240 class channels contiguous from partition 0 (compute-op partition ranges
must start at a 32-multiple; GPSIMD cannot run tensor ops in this lowering):
    A[128, HW] = [80 a0 cls | 48 a1 cls]
    B[127, HW] = [32 a1 cls | 80 a2 cls | 15 box rows (a,f)]
Encode (per group of 8 chunks):  q = rne_i32(4096*l)  on ACT (Copy, i32 out);
y = 128*q + (2^22+80-k):  A-half on DVE stt (int math), B-half on ACT
(Identity with per-partition f32 bias — exact, verified).  PE transposes each
chunk into PSUM [cell, 256]: class cols 0..240 ((a,k) stride 80), box cols
240..255; the per-scale B stationary carries `step` on its dx/dy diagonal
entries so box offsets arrive pre-scaled.  ONE DVE reduce per group gives
Y = 128*q* + 2^22 + rev*; box cols are staged to SBUF (ACT) so PSUM frees
after the reduce.  All box/cls/mask math then runs ONCE PER IMAGE over a
uniform [p, 30 chunks, 3, .] view (per-chunk anchor/grid constants), and one
output DMA per image writes [128, 540].

Quantization at 1/4096 flips argmax only for top-2 logit gaps < ~2.4e-4
(~1e-3 added rel err vs the 2e-2 gate on the reference distribution).
"""

import sys
from contextlib import ExitStack

import numpy as np

if "/opt/trn_rl_repo" not in sys.path:
    sys.path.insert(0, "/opt/trn_rl_repo")

NCORES = 8
B = 32
BLOC = B // NCORES  # images per core
CCOL = 256  # chunk stride in PSUM columns (240 class + 15 box + 1 pad)
PGRP = 8  # chunks per PSUM group tile
QS = 4096.0  # logit quantization scale
OFS = 1 << 22  # positivity offset, multiple of 128

# (name, H, W, HW, step, thresh, nchunks)
SCALES = [
    ("x13", 13, 13, 169, 32.0, 0.5, 2),
    ("x26", 26, 26, 676, 16.0, 0.5, 6),
    ("x52", 52, 52, 2704, 8.0, 0.9, 22),
]
ROWS_PER_B = sum(hw * 3 for _, _, _, hw, _, _, _ in SCALES)  # 10647
NCH = sum(nch for _, _, _, _, _, _, nch in SCALES)  # 30 chunks per image
CHUNK_BASE = [0, 2, 8]
WB = NCH * 18  # 540 out cols per image
OUT_FLAT = BLOC * 128 * WB
SOFF = {name: CHUNK_BASE[i] * 18 for i, (name, *_r) in enumerate(SCALES)}

# const tile (f32) column layout
C_IDA = 0  # [0:128] pure identity (A transposes)
C_IDB = 128  # [128+127*s : ...+127] per-scale B stationary with step diag
C_GXY = C_IDB + 3 * 127  # [509:569] per-chunk grid*step, [p, 30, 2]
C_ANC = C_GXY + NCH * 2  # [569:749] per-chunk anchors, [p, 30, 3, 2]
C_REVB = C_ANC + NCH * 6  # [749:750] B-tile encode bias (f32)
C_TOT = C_REVB + 1

_PROG_CACHE = {}


def _groups(nch):
    out = []
    g0 = 0
    while g0 < nch:
        out.append((g0, min(PGRP, nch - g0)))
        g0 += PGRP
    return out


def _build_program():
    import concourse.bacc as bacc
    import concourse.mybir as mybir
    from concourse.tile import TileContext

    f32 = mybir.dt.float32
    i32 = mybir.dt.int32
    AL = mybir.AluOpType
    AF = mybir.ActivationFunctionType
    AX = mybir.AxisListType

    nc = bacc.Bacc("TRN2", target_bir_lowering=False, debug=False)

    xin = {}
    for name, _, _, hw, _, _, _ in SCALES:
        xin[name] = nc.dram_tensor(
            name, [BLOC, 255, hw], f32, kind="ExternalInput"
        ).ap()
    c_f32 = nc.dram_tensor("c_f32", [128, C_TOT], f32, kind="ExternalInput").ap()
    c_rev = nc.dram_tensor("c_rev", [128, 1], i32, kind="ExternalInput").ap()
    out = nc.dram_tensor("out", [OUT_FLAT], f32, kind="ExternalOutput").ap()

    with TileContext(nc) as tc, ExitStack() as ctx:
        const = ctx.enter_context(tc.tile_pool(name="const", bufs=1))
        cf = const.tile([128, C_TOT], f32)
        nc.sync.dma_start(cf[:], c_f32[:])
        rev_t = const.tile([128, 1], i32)
        nc.sync.dma_start(rev_t[:], c_rev[:])
        identA = cf[:, C_IDA : C_IDA + 128]
        gxyS = cf[:, C_GXY : C_GXY + NCH * 2].rearrange("p (g j) -> p g j", j=2)
        anchg = cf[:, C_ANC : C_ANC + NCH * 6].rearrange(
            "p (g a j) -> p g a j", a=3, j=2
        )
        revB = cf[:, C_REVB : C_REVB + 1]

        in_pool = ctx.enter_context(tc.tile_pool(name="inp", bufs=2))
        q_pool = ctx.enter_context(tc.tile_pool(name="qp", bufs=2))
        ps_pool = ctx.enter_context(tc.tile_pool(name="ps", bufs=2, space="PSUM"))
        wk = ctx.enter_context(tc.tile_pool(name="wk", bufs=2))
        op = ctx.enter_context(tc.tile_pool(name="op", bufs=2))

        U = {}
        IMG = {}

        def emit_loads(b, s):
            name, Hh, Ww, HW, step, thresh, nch = SCALES[s]
            x = xin[name]
            A = in_pool.tile([128, HW], f32, tag=f"A{s}")
            Bt = in_pool.tile([127, HW], f32, tag=f"B{s}")
            nc.sync.dma_start(A[0:80, :], x[b, 5:85, :])
            nc.sync.dma_start(A[80:128, :], x[b, 90:138, :])
            nc.scalar.dma_start(Bt[0:32, :], x[b, 138:170, :])
            nc.scalar.dma_start(Bt[32:112, :], x[b, 175:255, :])
            # box rows: x[b, a*85+0:5, :] -> B[112+5a : 117+5a]
            # (one 2D DMA per anchor: partition-dim rearranges lower to a
            # linearized AP that clobbers neighboring partitions)
            for a in range(3):
                nc.gpsimd.dma_start(
                    Bt[112 + 5 * a : 117 + 5 * a, :],
                    x[b, 85 * a : 85 * a + 5, :],
                )
            U[(b, s)] = {"A": A, "B": Bt}

        def emit_compute(b, s):
            name, Hh, Ww, HW, step, thresh, nch = SCALES[s]
            u = U[(b, s)]
            A, Bt = u["A"], u["B"]
            if b not in IMG:
                Y_t = wk.tile([128, NCH * 3], f32, tag="Y")
                bc_t = wk.tile([128, NCH * 15], f32, tag="bc")
                IMG[b] = {"Y": Y_t, "bc": bc_t, "n": 0}
            IMG[b]["n"] += 1
            Y_t = IMG[b]["Y"]
            bc_t = IMG[b]["bc"]
            Yv = Y_t[:].rearrange("p (g a) -> p g a", a=3)
            cb = CHUNK_BASE[s]
            qA = q_pool.tile([128, HW], i32, tag=f"qA{s}")
            qB = q_pool.tile([127, HW], i32, tag=f"qB{s}")
            identB = cf[0:127, 0:127]
            gl = _groups(nch)
            # stage-batched: engine sequencers block on sem waits, so keep
            # each engine's stream free of cross-stage interleaving
            for g0, gch in gl:
                gw = min(HW, (g0 + gch) * 128) - g0 * 128
                cs = slice(g0 * 128, g0 * 128 + gw)
                # encode: q = rne_i32(4096*l)
                nc.scalar.activation(qA[:, cs], A[:, cs], AF.Copy, scale=QS)
                nc.scalar.activation(
                    qB[0:112, cs], Bt[0:112, cs], AF.Copy, scale=QS
                )
            for g0, gch in gl:
                gw = min(HW, (g0 + gch) * 128) - g0 * 128
                cs = slice(g0 * 128, g0 * 128 + gw)
                # y = 128*q + rev: A-half int stt on DVE, B-half on ACT
                # (Identity, per-partition f32 bias; exact for |y| < 2^24)
                nc.vector.scalar_tensor_tensor(
                    out=A[:, cs],
                    in0=qA[:, cs],
                    scalar=128,
                    in1=rev_t[:, 0:1].broadcast_to([128, gw]),
                    op0=AL.mult,
                    op1=AL.add,
                )
                nc.scalar.activation(
                    Bt[0:112, cs],
                    qB[0:112, cs],
                    AF.Identity,
                    bias=revB[0:112, :],
                    scale=128.0,
                )
            Ps = []
            for g0, gch in gl:
                P = ps_pool.tile([128, PGRP * CCOL], f32, tag="P")
                Ps.append(P)
                for c in range(gch):
                    gc = g0 + c
                    cells = min(128, HW - gc * 128)
                    col = gc * 128
                    fo = c * CCOL
                    # both transposes use tile_position (0,0): mixing
                    # base-0/base-64 positions on one PSUM bank is fatal
                    nc.tensor.transpose(
                        P[0:cells, fo : fo + 128],
                        A[:, col : col + cells],
                        identA,
                    )
                    nc.tensor.transpose(
                        P[0:cells, fo + 128 : fo + 255],
                        Bt[:, col : col + cells],
                        identB,
                    )
            for (g0, gch), P in zip(gl, Ps):
                Pg = P[:, :].rearrange("p (g f) -> p g f", f=CCOL)[:, 0:gch]
                # classes at cols 0..240: (a,k) stride 80
                P4c = Pg[:, :, 0:240].rearrange("p g (a k) -> p g a k", k=80)
                gs = slice(cb + g0, cb + g0 + gch)
                # ONE reduce: Y = max_k y  (argmax encoded in low bits)
                nc.vector.tensor_reduce(
                    out=Yv[:, gs, :], in_=P4c, axis=AX.X, op=AL.max
                )
                # stage box cols to SBUF: P frees after the reduce; box math
                # runs once per image
                nc.scalar.activation(
                    out=bc_t[
                        :, (cb + g0) * 15 : (cb + g0 + gch) * 15
                    ].rearrange("p (g f) -> p g f", f=15),
                    in_=Pg[:, :, 240:255],
                    func=AF.Copy,
                )

        def emit_tail(b):
            # per-image box/cls/mask math over uniform [p, 30, 3, .] views
            Y_t = IMG[b]["Y"]
            bc_t = IMG.pop(b)["bc"]
            bcv = bc_t[:].rearrange("p (g a f) -> p g a f", a=3, f=5)
            Ob = op.tile([128, WB], f32, tag="Ob")
            O4 = Ob[:].rearrange("p (g a f) -> p g a f", a=3, f=6)
            O3 = Ob[:].rearrange("p (ga f) -> p ga f", f=6)
            M_t = wk.tile([128, NCH * 3], f32, tag="M")
            fi_t = wk.tile([128, NCH * 3], i32, tag="fi")
            ff_t = wk.tile([128, NCH * 3], f32, tag="ff")
            wh_t = wk.tile([128, 2 * NCH * 3], f32, tag="wh")
            whv = wh_t[:].rearrange("p (j g a) -> p j g a", j=2, a=3)
            u_t = wk.tile([128, 2 * NCH * 3], f32, tag="u")
            uv = u_t[:].rearrange("p (j g a) -> p j g a", j=2, a=3)

            Mv = M_t[:].rearrange("p (g a) -> p g a", a=3)
            nc.vector.tensor_scalar(
                out=Mv[:, 0:8, :],
                in0=bcv[:, 0:8, :, 0],
                scalar1=0.5,
                scalar2=None,
                op0=AL.is_gt,
            )
            nc.vector.tensor_scalar(
                out=Mv[:, 8:30, :],
                in0=bcv[:, 8:30, :, 0],
                scalar1=0.9,
                scalar2=None,
                op0=AL.is_gt,
            )
            # conf: plain copy; the final mask-mult zeroes it where needed
            nc.scalar.activation(
                out=O4[:, :, :, 0], in_=bcv[:, :, :, 0], func=AF.Copy
            )
            for j in range(2):  # j=0: x/w, j=1: y/h
                nc.scalar.activation(
                    out=whv[:, j], in_=bcv[:, :, :, 3 + j], func=AF.Exp
                )
                nc.vector.tensor_tensor(
                    out=whv[:, j],
                    in0=whv[:, j],
                    in1=anchg[:, :, :, j],
                    op=AL.mult,
                )
                # u = dx*step + gx*step, per scale (step differs)
                for s2, (_n2, _h2, _w2, _hw2, step2, _t2, _nc2) in enumerate(
                    SCALES
                ):
                    sl = slice(CHUNK_BASE[s2], CHUNK_BASE[s2] + SCALES[s2][6])
                    w2 = sl.stop - sl.start
                    nc.vector.scalar_tensor_tensor(
                        out=uv[:, j, sl],
                        in0=bcv[:, sl, :, 1 + j],
                        scalar=step2,
                        in1=gxyS[:, sl, j]
                        .unsqueeze(2)
                        .broadcast_to([128, w2, 3]),
                        op0=AL.mult,
                        op1=AL.add,
                    )
                nc.vector.scalar_tensor_tensor(
                    out=O4[:, :, :, 1 + j],
                    in0=whv[:, j],
                    scalar=-0.5,
                    in1=uv[:, j],
                    op0=AL.mult,
                    op1=AL.add,
                )
                nc.vector.tensor_tensor(
                    out=O4[:, :, :, 3 + j],
                    in0=O4[:, :, :, 1 + j],
                    in1=whv[:, j],
                    op=AL.add,
                )
            # cls = 80 - (Y mod 128): trunc via i32 cast with -0.499 bias
            nc.vector.tensor_scalar(
                out=fi_t[:],
                in0=Y_t[:],
                scalar1=1.0 / 128.0,
                scalar2=-0.499,
                op0=AL.mult,
                op1=AL.add,
            )
            nc.vector.tensor_scalar(
                out=ff_t[:], in0=fi_t[:], scalar1=0, scalar2=None, op0=AL.add
            )
            nc.vector.scalar_tensor_tensor(
                out=O3[:, :, 5],
                in0=ff_t[:],
                scalar=128.0,
                in1=Y_t[:],
                op0=AL.mult,
                op1=AL.subtract,
            )
            nc.vector.tensor_scalar(
                out=O3[:, :, 5],
                in0=O3[:, :, 5],
                scalar1=80.0,
                scalar2=None,
                op0=AL.add,
            )
            # zero masked cells
            nc.vector.tensor_tensor(
                out=O3,
                in0=O3,
                in1=M_t[:].unsqueeze(2).broadcast_to([128, NCH * 3, 6]),
                op=AL.mult,
            )
            dst = out[b * 128 * WB : (b + 1) * 128 * WB].rearrange(
                "(p w) -> p w", w=WB
            )
            nc.scalar.dma_start(dst, Ob[:, :])

        # software pipeline: loads(i) | compute(i-1) | tail after each image
        units = (
            [(0, 0), (0, 1), (0, 2)]
            + [(b, s) for b in range(1, BLOC - 1) for s in (2, 0, 1)]
            + [(BLOC - 1, s) for s in (2, 1, 0)]
        )
        n = len(units)
        for i in range(n + 1):
            if i < n:
                emit_loads(*units[i])
            if 1 <= i <= n:
                bb, ss = units[i - 1]
                emit_compute(bb, ss)
                if IMG[bb]["n"] == len(SCALES):
                    emit_tail(bb)
    nc.compile()
    return nc


def _host_constants(anchors):
    anchors = np.asarray(anchors, dtype=np.float32).reshape(3, 3, 2)
    rev = np.zeros((128, 1), dtype=np.int32)
    for p in range(0, 80):
        rev[p, 0] = OFS + 80 - p  # a0 cls k = p
    for p in range(80, 128):
        rev[p, 0] = OFS + 80 - (p - 80)  # a1 cls k = p-80 (0..48)
    cf = np.zeros((128, C_TOT), dtype=np.float32)
    cf[:, 0:128] = np.eye(128, dtype=np.float32)
    gxy = np.zeros((128, NCH, 2), dtype=np.float32)
    ang = np.zeros((128, NCH, 3, 2), dtype=np.float32)
    for s, (name, Hh, Ww, HW, step, thresh, nch) in enumerate(SCALES):
        for c in range(nch):
            g = CHUNK_BASE[s] + c
            ang[:, g] = anchors[s][None, :, :]
            for p in range(128):
                hw = c * 128 + p
                if hw < HW:
                    gxy[p, g, 0] = (hw % Ww) * step
                    gxy[p, g, 1] = (hw // Ww) * step
    cf[:, C_GXY : C_GXY + NCH * 2] = gxy.reshape(128, NCH * 2)
    cf[:, C_ANC : C_ANC + NCH * 6] = ang.reshape(128, NCH * 6)
    revb = np.zeros((128, 1), dtype=np.float32)
    for p in range(0, 32):
        revb[p, 0] = OFS + 80 - (p + 48)  # a1 cls k = p+48
    for p in range(32, 112):
        revb[p, 0] = OFS + 80 - (p - 32)  # a2 cls k = p-32
    cf[:, C_REVB : C_REVB + 1] = revb
    return {"c_f32": cf, "c_rev": rev}


def kernel(output13, output26, output52, anchors):
    from concourse.bass_utils import run_bass_kernel_spmd

    if "nc" not in _PROG_CACHE:
        _PROG_CACHE["nc"] = _build_program()
    nc = _PROG_CACHE["nc"]

    consts = _host_constants(np.asarray(anchors, dtype=np.float32))
    xs = {
        "x13": np.asarray(output13, dtype=np.float32).reshape(B, 255, 169),
        "x26": np.asarray(output26, dtype=np.float32).reshape(B, 255, 676),
        "x52": np.asarray(output52, dtype=np.float32).reshape(B, 255, 2704),
    }
    in_maps = []
    for i in range(NCORES):
        m = dict(consts)
        for k, v in xs.items():
            m[k] = np.ascontiguousarray(v[i * BLOC : (i + 1) * BLOC])
        in_maps.append(m)

    _PROG_CACHE["in_maps"] = in_maps
    res = run_bass_kernel_spmd(nc, in_maps, core_ids=list(range(NCORES)))

    full = np.zeros((B * ROWS_PER_B, 6), np.float32)
    scale_full_base = [0, B * 169 * 3, B * 169 * 3 + B * 676 * 3]
    for i in range(NCORES):
        o = np.asarray(res.results[i]["out"]).reshape(BLOC, 128, WB)
        for b in range(BLOC):
            for s, (name, Hh, Ww, HW, step, thresh, nch) in enumerate(SCALES):
                so = SOFF[name]
                seg = o[b][:, so : so + nch * 18].reshape(128, nch, 3, 6)
                rows = seg.transpose(1, 0, 2, 3).reshape(nch * 128 * 3, 6)
                gb = scale_full_base[s] + (i * BLOC + b) * HW * 3
                full[gb : gb + HW * 3] = rows[: HW * 3]
    return full
